# revision 15
# baseline (speedup 1.0000x reference)
"""Trainium2 Bass kernel for few-shot video retrieval (bidirectional chamfer
distance to class prototypes, global frame-level + segment-level, fused).

Contract: kernel(**inputs) takes the FULL unsharded inputs (numpy) and returns
the full outputs (tuple of 4 [4096, 64] float32 arrays), matching reference().

Sharding: data-parallel over the query axis across 8 NeuronCores; prototypes
(computed on host, like the norm factors) replicated. Gather + fusion on host.

Device-side algorithm per core (512 queries = 4 slices of 128):
  - host pre-normalizes every query frame (x64) and every prototype frame
    (x16) in f32, then casts to fp8 e4m3 -> all PSUM results are 1024*sim
    with a single constant drain scale; no per-(q,t) norm factors on device
  - main sims GEMM: queries stationary (d-major), protoT moving, fp8
    DoubleRow (256-deep contraction), output [q, (k, ts)] with ts innermost
  - chamfer: dir0 (max over ts) = two grouped tensor_reduce ops (contiguous
    innermost axis, 2x bf16); dir1 (max over tq) = pairwise bf16 max TTs that
    pipeline with the PSUM drains; sums via strided reduces
  - segments: 3 separate GEMM groups (one per support window v), stationary
    = seg prototypes [d, k] so outputs land k-major on partitions 0-63;
    chamfer trees split between DVE and GpSimd
  - fusion softmax/exp + final gather/transpose on host
"""

import sys

sys.path.insert(0, "/opt/trn_rl_repo")

import numpy as np
import ml_dtypes
from contextlib import ExitStack

import concourse.bass as bass
import concourse.bacc as bacc
import concourse.tile as tile
from concourse import mybir
from concourse.bass_utils import run_bass_kernel_spmd

# ---------------------------------------------------------------- problem dims
S, Q, T, D = 256, 4096, 8, 1024
K = 64                      # classes
NCORES = 8
QPC = Q // NCORES           # 512 queries per core
G = QPC // 128              # 4 query-slices of 128 per core
DJ = 4                      # 4 DoubleRow chunks (256-deep)
NW = 3                      # segment windows
WINDOWS = ((0, 4), (2, 6), (4, 8))
QSC = 64.0                  # query fp8 scale (host-normalized frames)
PSC = 16.0                  # prototype fp8 scale
ISC = 1.0 / (QSC * PSC)     # drain scale: PSUM value = 1024 * sim

F32 = mybir.dt.float32
BF16 = mybir.dt.bfloat16
F8 = mybir.dt.float8e4
AF = mybir.ActivationFunctionType
ALU = mybir.AluOpType
AX = mybir.AxisListType
DR = mybir.MatmulPerfMode.DoubleRow

NP_F8 = ml_dtypes.float8_e4m3


# ---------------------------------------------------------------- bass kernel
def build_nc():
    nc = bacc.Bacc("TRN2", target_bir_lowering=False, debug=False,
                   num_devices=NCORES)

    # d-major normalized queries: [g][p(d%128)][(t, j, o, q)] (t-major so
    # each tq matmul group depends only on its own 1KB/partition slice)
    tf = nc.dram_tensor("tf", [G, 128, DJ * 2 * T * 128], F8,
                        kind="ExternalInput")
    # d-major normalized query segments: [g][p][(j, o, w, q)]
    tsg = nc.dram_tensor("tsg", [G, 128, DJ * 2 * NW * 128], F8,
                         kind="ExternalInput")
    # d-major normalized frame prototypes: [p][(j, o, k, s)]
    pT = nc.dram_tensor("pT", [128, DJ * 2 * K * T], F8, kind="ExternalInput")
    # d-major normalized segment prototypes: [p][(j, o, v, k)]
    sT = nc.dram_tensor("sT", [128, DJ * 2 * NW * K], F8,
                        kind="ExternalInput")

    # outputs: -global_dist q-major; -(s2q|q2s) k-major
    oglo = nc.dram_tensor("oglo", [128, G * K], F32, kind="ExternalOutput")
    oseg = nc.dram_tensor("oseg", [64, G * 2 * 128], F32,
                          kind="ExternalOutput")

    NWARM = 12

    with tile.TileContext(nc) as tc, ExitStack() as ctx:
        const = ctx.enter_context(tc.tile_pool(name="const", bufs=1))
        persist = ctx.enter_context(tc.tile_pool(name="persist", bufs=1))
        simpool = ctx.enter_context(tc.tile_pool(name="simpool", bufs=2))
        segpool = ctx.enter_context(tc.tile_pool(name="segpool", bufs=2))
        work = ctx.enter_context(tc.tile_pool(name="work", bufs=2))

        # ---------------- PE warmup burst (HAM ramp while inputs stream in);
        # memset rides the vector queue so gpsimd's first op is a DMA issue
        wz = const.tile([128, 256], F8)
        nc.vector.memset(wz[:], 0)
        with tc.tile_pool(name="psW", bufs=1, space="PSUM") as psW:
            wps = psW.tile([128, 256], F32)
            for _ in range(NWARM):
                nc.tensor.matmul(wps[:], wz[:, 0:128], wz[:], start=True,
                                 stop=True)

        # ---------------- input DMAs split across three engine queues so
        # the head issues run in parallel right after the preambles
        pT_t = const.tile([128, DJ * 2 * K * T], F8)
        for h in range(2):
            cols = slice(h * 2048, (h + 1) * 2048)
            nc.sync.dma_start(pT_t[:, cols], pT[:, cols])
        tf_t = []
        tsg_t = []
        for g in range(G):
            tf_t.append(const.tile([128, DJ * 2 * T * 128], F8,
                                   name=f"tf{g}"))
            tsg_t.append(const.tile([128, DJ * 2 * NW * 128], F8,
                                    name=f"tsg{g}"))
        for h in range(4):
            cols = slice(h * 2048, (h + 1) * 2048)
            nc.gpsimd.dma_start(tf_t[0][:, cols], tf[0][:, cols])
        sT_t = const.tile([128, DJ * 2 * NW * K], F8)
        nc.scalar.dma_start(sT_t[:], sT[:])
        nc.scalar.dma_start(tsg_t[0][:], tsg[0])
        for g in range(1, G):
            for h in range(2):
                cols = slice(h * 4096, (h + 1) * 4096)
                nc.gpsimd.dma_start(tf_t[g][:, cols], tf[g][:, cols])
        for g in range(1, G):
            nc.gpsimd.dma_start(tsg_t[g][:], tsg[g])

        pT_v = pT_t[:].rearrange("p (j o k s) -> p j o k s", j=DJ, o=2, k=K)
        sT_v = sT_t[:].rearrange("p (j o v k) -> p j o v k", j=DJ, o=2, v=NW)

        obuf_glo = persist.tile([128, G * K], F32)
        obuf_seg = persist.tile([64, G * 2 * 128], F32)

        psM = ctx.enter_context(tc.tile_pool(name="psM", bufs=3,
                                             space="PSUM"))
        psS = ctx.enter_context(tc.tile_pool(name="psS", bufs=1,
                                             space="PSUM"))

        def emit_main(g):
            tf_v = tf_t[g][:].rearrange("p (t j o q) -> p t j o q", t=T,
                                        j=DJ, o=2)
            simcp = simpool.tile([128, T * K * T], BF16, tag="simcp")
            pmax = simpool.tile([128, 4 * K * T], BF16, tag="pmax")
            Lh = simpool.tile([128, 2 * 4 * K * 4], BF16, tag="Lh")
            for tq in range(T):
                mp = psM.tile([128, K * T], F32, tag="mp")
                for j in range(DJ):
                    nc.tensor.matmul(mp[:], tf_v[:, tq, j, :, :],
                                     pT_v[:, j], start=(j == 0),
                                     stop=(j == DJ - 1), perf_mode=DR)
                nc.scalar.activation(simcp[:, tq * 512:(tq + 1) * 512],
                                     mp[:], AF.Copy, scale=ISC)
                if tq % 2 == 1:
                    i = tq // 2
                    nc.vector.tensor_tensor(
                        pmax[:, i * 512:(i + 1) * 512],
                        simcp[:, (tq - 1) * 512:tq * 512],
                        simcp[:, tq * 512:(tq + 1) * 512], ALU.max)
                if tq == 3 or tq == 7:
                    # dir1 half-merge + dir0 s-halving L1 (hide under MMs)
                    h = tq // 4
                    Th = work.tile([128, 512], BF16, tag=f"T{h}")
                    nc.vector.tensor_tensor(
                        Th[:], pmax[:, h * 1024:h * 1024 + 512],
                        pmax[:, h * 1024 + 512:h * 1024 + 1024], ALU.max)
                    if h == 0:
                        T1 = Th
                    else:
                        T2 = Th
                    sh = simcp[:, h * 2048:(h + 1) * 2048].rearrange(
                        "p (tk s) -> p tk s", s=T)
                    nc.vector.tensor_tensor(
                        Lh[:, h * 1024:(h + 1) * 1024].rearrange(
                            "p (tk s) -> p tk s", s=4),
                        sh[:, :, 0:4], sh[:, :, 4:8], ALU.max)
            Rm = work.tile([128, 512], BF16, tag="Rm")
            nc.vector.tensor_tensor(Rm[:], T1[:], T2[:], ALU.max)
            msum = work.tile([128, K], F32, tag="msum")
            nc.vector.tensor_reduce(msum[:],
                                    Rm[:].rearrange("p (k s) -> p k s", k=K),
                                    axis=AX.X, op=ALU.add)
            # dir0 L2/L3: (h,t,k,s4) -> (h,t,k)
            L2 = work.tile([128, 1024], BF16, tag="L2")
            lhv = Lh[:].rearrange("p (tk s) -> p tk s", s=4)
            nc.vector.tensor_tensor(
                L2[:].rearrange("p (tk s) -> p tk s", s=2),
                lhv[:, :, 0:2], lhv[:, :, 2:4], ALU.max)
            Am = work.tile([128, 512], BF16, tag="Am")
            l2v = L2[:].rearrange("p (tk s) -> p tk s", s=2)
            nc.vector.tensor_tensor(Am[:].rearrange("p (tk s) -> p tk s",
                                                    s=1),
                                    l2v[:, :, 0:1], l2v[:, :, 1:2], ALU.max)
            # asum tree over t: Am layout (h2, t4, k64)
            h1 = work.tile([128, 256], BF16, tag="h1")
            nc.vector.tensor_tensor(h1[:], Am[:, 0:256], Am[:, 256:512],
                                    ALU.add)
            h2 = work.tile([128, 128], BF16, tag="h2")
            nc.vector.tensor_tensor(h2[:], h1[:, 0:128], h1[:, 128:256],
                                    ALU.add)
            asum = work.tile([128, K], F32, tag="asum")
            nc.vector.tensor_tensor(asum[:], h2[:, 0:64], h2[:, 64:128],
                                    ALU.add)
            nc.vector.scalar_tensor_tensor(
                obuf_glo[:, g * K:(g + 1) * K], in0=asum[:], scalar=-16.0,
                in1=msum[:], op0=ALU.add, op1=ALU.add)
            nc.gpsimd.dma_start(oglo[:, g * K:(g + 1) * K],
                                obuf_glo[:, g * K:(g + 1) * K])

        def emit_seg(g):
            tsg_v = tsg_t[g][:].rearrange("p (j o w q) -> p j o w q", j=DJ,
                                          o=2, w=NW)
            segs = segpool.tile([64, NW * NW * 128], BF16, tag="segs")
            for v in range(NW):
                sp = psS.tile([64, NW * 128], F32, tag=f"sv{v}")
                for j in range(DJ):
                    nc.tensor.matmul(sp[:], sT_v[:, j, :, v, :],
                                     tsg_v[:, j], start=(j == 0),
                                     stop=(j == DJ - 1), perf_mode=DR)
                nc.scalar.activation(segs[:, v * 384:(v + 1) * 384], sp[:],
                                     AF.Copy, scale=ISC)
            # q2s = sum_w max_v  (contiguous 384-wide maxes over v-slices)
            m01 = work.tile([64, NW * 128], BF16, tag="m01")
            nc.vector.tensor_tensor(m01[:], segs[:, 0:384], segs[:, 384:768],
                                    ALU.max)
            m012 = work.tile([64, NW * 128], BF16, tag="m012")
            nc.vector.tensor_tensor(m012[:], m01[:], segs[:, 768:1152],
                                    ALU.max)
            ws = work.tile([64, 128], BF16, tag="ws")
            nc.vector.tensor_tensor(ws[:], m012[:, 0:128], m012[:, 128:256],
                                    ALU.add)
            nc.vector.scalar_tensor_tensor(
                obuf_seg[:, g * 256 + 128:g * 256 + 256], in0=ws[:],
                scalar=-3.0, in1=m012[:, 256:384], op0=ALU.add, op1=ALU.add)
            # s2q = sum_v max_w  (strided views over w, v in the free dim)
            vv = segs[:].rearrange("p (v w q) -> p v w q", v=NW, w=NW)
            W1 = work.tile([64, NW * 128], BF16, tag="W1")
            w1v = W1[:].rearrange("p (v q) -> p v q", v=NW)
            nc.vector.tensor_tensor(w1v, vv[:, :, 0, :], vv[:, :, 1, :],
                                    ALU.max)
            Wm = work.tile([64, NW * 128], BF16, tag="Wm")
            wmv = Wm[:].rearrange("p (v q) -> p v q", v=NW)
            nc.vector.tensor_tensor(wmv, w1v, vv[:, :, 2, :], ALU.max)
            vs = work.tile([64, 128], BF16, tag="vs")
            nc.vector.tensor_tensor(vs[:], Wm[:, 0:128], Wm[:, 128:256],
                                    ALU.add)
            nc.vector.scalar_tensor_tensor(
                obuf_seg[:, g * 256:g * 256 + 128], in0=vs[:], scalar=-3.0,
                in1=Wm[:, 256:384], op0=ALU.add, op1=ALU.add)
            nc.gpsimd.dma_start(oseg[:, g * 256:(g + 1) * 256],
                                obuf_seg[:, g * 256:(g + 1) * 256])

        for g in range(G):
            emit_main(g)
        for g in range(G):
            emit_seg(g)

    nc.compile()
    return nc


_NC_CACHE = None


def _get_nc():
    global _NC_CACHE
    if _NC_CACHE is None:
        _NC_CACHE = build_nc()
    return _NC_CACHE


# ------------------------------------------------------------------ host side
def _norm8(x, scale):
    n = np.sqrt((x * x).sum(-1, keepdims=True))
    n = np.maximum(n, 1e-12)
    return (scale * x / n).astype(NP_F8)


def build_in_maps(support_features, target_features, support_labels,
                  logit_scale, fusion_logits):
    support_features = np.asarray(support_features, dtype=np.float32)
    target_features = np.asarray(target_features, dtype=np.float32)
    support_labels = np.asarray(support_labels, dtype=np.int32)

    # ---- prototypes (exact f32 scatter-mean, normalized, x16, fp8)
    proto = np.zeros((K, T, D), np.float32)
    cnt = np.zeros((K,), np.float32)
    np.add.at(proto, support_labels % K, support_features)
    np.add.at(cnt, support_labels % K, 1.0)
    proto /= cnt[:, None, None]
    p8 = _norm8(proto, PSC)                                   # [K, T, D]
    segp = np.stack([proto[:, s:e].sum(1) for s, e in WINDOWS], 1)
    sp8 = _norm8(segp, PSC)                                   # [K, NW, D]

    # pT: [p][(j,o,k,s)]
    pT_h = np.ascontiguousarray(
        p8.reshape(K, T, DJ, 2, 128).transpose(4, 2, 3, 0, 1)
    ).reshape(128, DJ * 2 * K * T)
    # sT: [p][(j,o,v,k)]
    sT_h = np.ascontiguousarray(
        sp8.reshape(K, NW, DJ, 2, 128).transpose(4, 2, 3, 1, 0)
    ).reshape(128, DJ * 2 * NW * K)

    # ---- queries: normalized x64, fp8, d-major
    q8 = _norm8(target_features, QSC)                         # [Q, T, D]
    segq = np.stack([target_features[:, s:e].sum(1) for s, e in WINDOWS], 1)
    sq8 = _norm8(segq, QSC)                                   # [Q, NW, D]

    in_maps = []
    for c in range(NCORES):
        x8 = q8[c * QPC:(c + 1) * QPC]
        tf_h = np.ascontiguousarray(
            x8.reshape(G, 128, T, DJ, 2, 128).transpose(0, 5, 2, 3, 4, 1)
        ).reshape(G, 128, DJ * 2 * T * 128)
        s8 = sq8[c * QPC:(c + 1) * QPC]
        tsg_h = np.ascontiguousarray(
            s8.reshape(G, 128, NW, DJ, 2, 128).transpose(0, 5, 3, 4, 2, 1)
        ).reshape(G, 128, DJ * 2 * NW * 128)
        in_maps.append({"tf": tf_h, "tsg": tsg_h, "pT": pT_h, "sT": sT_h})
    return in_maps


def kernel(support_features, target_features, support_labels, logit_scale,
           fusion_logits):
    logit_scale = np.asarray(logit_scale, dtype=np.float32)
    fusion_logits = np.asarray(fusion_logits, dtype=np.float32)
    in_maps = build_in_maps(support_features, target_features, support_labels,
                            logit_scale, fusion_logits)
    nc = _get_nc()
    res = run_bass_kernel_spmd(nc, in_maps, core_ids=list(range(NCORES)))

    glo = np.empty((Q, K), np.float32)
    s2q = np.empty((Q, K), np.float32)
    q2s = np.empty((Q, K), np.float32)
    for c in range(NCORES):
        og = np.asarray(res.results[c]["oglo"]).reshape(128, G, K)
        glo[c * QPC:(c + 1) * QPC] = og.transpose(1, 0, 2).reshape(QPC, K)
        os_ = np.asarray(res.results[c]["oseg"]).reshape(64, G, 2, 128)
        # [k, g, which, q] -> [g, q, k]
        s2q[c * QPC:(c + 1) * QPC] = os_[:, :, 0].transpose(1, 2, 0).reshape(
            QPC, K)
        q2s[c * QPC:(c + 1) * QPC] = os_[:, :, 1].transpose(1, 2, 0).reshape(
            QPC, K)

    e = np.exp(fusion_logits - fusion_logits.max())
    fw = (e / e.sum()) * np.exp(logit_scale)
    fused = fw[0] * glo + fw[1] * s2q + fw[2] * q2s
    return (fused.astype(np.float32), glo, s2q, q2s)


if __name__ == "__main__":
    rng = np.random.default_rng(0)
    ins = {
        "support_features": rng.standard_normal((S, T, D), dtype=np.float32),
        "target_features": rng.standard_normal((Q, T, D), dtype=np.float32),
        "support_labels": (np.arange(S) % K).astype(np.int32),
        "logit_scale": np.float32(0.0),
        "fusion_logits": np.zeros(3, np.float32),
    }
    outs = kernel(**ins)
    for o in outs:
        print(o.shape, o.dtype, float(o.mean()))


# revision 17
# speedup vs baseline: 1.0272x; 1.0272x over previous
"""Trainium2 Bass kernel for few-shot video retrieval (bidirectional chamfer
distance to class prototypes, global frame-level + segment-level, fused).

Contract: kernel(**inputs) takes the FULL unsharded inputs (numpy) and returns
the full outputs (tuple of 4 [4096, 64] float32 arrays), matching reference().

Sharding: data-parallel over the query axis across 8 NeuronCores; prototypes
(computed on host, like the norm factors) replicated. Gather + fusion on host.

Device-side algorithm per core (512 queries = 4 slices of 128):
  - host pre-normalizes every query frame (x64) and every prototype frame
    (x16) in f32, then casts to fp8 e4m3 -> all PSUM results are 1024*sim
    with a single constant drain scale; no per-(q,t) norm factors on device
  - main sims GEMM: queries stationary (d-major), protoT moving, fp8
    DoubleRow (256-deep contraction), output [q, (k, ts)] with ts innermost
  - chamfer: dir0 (max over ts) = two grouped tensor_reduce ops (contiguous
    innermost axis, 2x bf16); dir1 (max over tq) = pairwise bf16 max TTs that
    pipeline with the PSUM drains; sums via strided reduces
  - segments: 3 separate GEMM groups (one per support window v), stationary
    = seg prototypes [d, k] so outputs land k-major on partitions 0-63;
    chamfer trees split between DVE and GpSimd
  - fusion softmax/exp + final gather/transpose on host
"""

import sys

sys.path.insert(0, "/opt/trn_rl_repo")

import numpy as np
import ml_dtypes
from contextlib import ExitStack

import concourse.bass as bass
import concourse.bacc as bacc
import concourse.tile as tile
from concourse import mybir
from concourse.bass_utils import run_bass_kernel_spmd

# ---------------------------------------------------------------- problem dims
S, Q, T, D = 256, 4096, 8, 1024
K = 64                      # classes
NCORES = 8
QPC = Q // NCORES           # 512 queries per core
G = QPC // 128              # 4 query-slices of 128 per core
DJ = 4                      # 4 DoubleRow chunks (256-deep)
NW = 3                      # segment windows
WINDOWS = ((0, 4), (2, 6), (4, 8))
QSC = 64.0                  # query fp8 scale (host-normalized frames)
PSC = 16.0                  # prototype fp8 scale
ISC = 1.0 / (QSC * PSC)     # drain scale: PSUM value = 1024 * sim

F32 = mybir.dt.float32
BF16 = mybir.dt.bfloat16
F8 = mybir.dt.float8e4
AF = mybir.ActivationFunctionType
ALU = mybir.AluOpType
AX = mybir.AxisListType
DR = mybir.MatmulPerfMode.DoubleRow

NP_F8 = ml_dtypes.float8_e4m3


# ---------------------------------------------------------------- bass kernel
def build_nc():
    nc = bacc.Bacc("TRN2", target_bir_lowering=False, debug=False,
                   num_devices=NCORES)

    # d-major normalized queries: [g][p(d%128)][(t, j, o, q)] (t-major so
    # each tq matmul group depends only on its own 1KB/partition slice)
    tf = nc.dram_tensor("tf", [G, 128, DJ * 2 * T * 128], F8,
                        kind="ExternalInput")
    # d-major normalized query segments: [g][p][(j, o, w, q)]
    tsg = nc.dram_tensor("tsg", [G, 128, DJ * 2 * NW * 128], F8,
                         kind="ExternalInput")
    # d-major normalized frame prototypes: [p][(j, o, k, s)]
    pT = nc.dram_tensor("pT", [128, DJ * 2 * K * T], F8, kind="ExternalInput")
    # d-major normalized segment prototypes: [p][(j, o, v, k)]
    sT = nc.dram_tensor("sT", [128, DJ * 2 * NW * K], F8,
                        kind="ExternalInput")

    # outputs: -global_dist q-major; -(s2q|q2s) k-major
    oglo = nc.dram_tensor("oglo", [128, G * K], F32, kind="ExternalOutput")
    oseg = nc.dram_tensor("oseg", [64, G * 2 * 128], F32,
                          kind="ExternalOutput")

    NWARM = 12

    with tile.TileContext(nc) as tc, ExitStack() as ctx:
        const = ctx.enter_context(tc.tile_pool(name="const", bufs=1))
        persist = ctx.enter_context(tc.tile_pool(name="persist", bufs=1))
        simpool = ctx.enter_context(tc.tile_pool(name="simpool", bufs=2))
        segpool = ctx.enter_context(tc.tile_pool(name="segpool", bufs=2))
        work = ctx.enter_context(tc.tile_pool(name="work", bufs=2))

        # ---------------- PE warmup burst (HAM ramp while inputs stream in)
        wz = const.tile([128, 256], F8)
        nc.gpsimd.memset(wz[:], 0)
        with tc.tile_pool(name="psW", bufs=1, space="PSUM") as psW:
            wps = psW.tile([128, 256], F32)
            for _ in range(NWARM):
                nc.tensor.matmul(wps[:], wz[:, 0:128], wz[:], start=True,
                                 stop=True)

        # ---------------- input DMAs (gpsimd queue wakes earliest; issue
        # order = priority order)
        pT_t = const.tile([128, DJ * 2 * K * T], F8)
        for h in range(2):
            cols = slice(h * 2048, (h + 1) * 2048)
            nc.gpsimd.dma_start(pT_t[:, cols], pT[:, cols])
        tf_t = []
        tsg_t = []
        for g in range(G):
            tf_t.append(const.tile([128, DJ * 2 * T * 128], F8,
                                   name=f"tf{g}"))
            tsg_t.append(const.tile([128, DJ * 2 * NW * 128], F8,
                                    name=f"tsg{g}"))
        for h in range(4):
            cols = slice(h * 2048, (h + 1) * 2048)
            nc.gpsimd.dma_start(tf_t[0][:, cols], tf[0][:, cols])
        for g in range(1, G):
            for h in range(2):
                cols = slice(h * 4096, (h + 1) * 4096)
                nc.gpsimd.dma_start(tf_t[g][:, cols], tf[g][:, cols])
        sT_t = const.tile([128, DJ * 2 * NW * K], F8)
        nc.gpsimd.dma_start(sT_t[:], sT[:])
        for g in range(G):
            nc.gpsimd.dma_start(tsg_t[g][:], tsg[g])

        pT_v = pT_t[:].rearrange("p (j o k s) -> p j o k s", j=DJ, o=2, k=K)
        sT_v = sT_t[:].rearrange("p (j o v k) -> p j o v k", j=DJ, o=2, v=NW)

        obuf_glo = persist.tile([128, G * K], F32)
        obuf_seg = persist.tile([64, G * 2 * 128], F32)

        psM = ctx.enter_context(tc.tile_pool(name="psM", bufs=3,
                                             space="PSUM"))
        psS = ctx.enter_context(tc.tile_pool(name="psS", bufs=1,
                                             space="PSUM"))

        def emit_main(g):
            tf_v = tf_t[g][:].rearrange("p (t j o q) -> p t j o q", t=T,
                                        j=DJ, o=2)
            simcp = simpool.tile([128, T * K * T], BF16, tag="simcp")
            pmax = simpool.tile([128, 4 * K * T], BF16, tag="pmax")
            Lh = simpool.tile([128, 2 * 4 * K * 4], BF16, tag="Lh")
            for tq in range(T):
                mp = psM.tile([128, K * T], F32, tag="mp")
                for j in range(DJ):
                    nc.tensor.matmul(mp[:], tf_v[:, tq, j, :, :],
                                     pT_v[:, j], start=(j == 0),
                                     stop=(j == DJ - 1), perf_mode=DR)
                nc.scalar.activation(simcp[:, tq * 512:(tq + 1) * 512],
                                     mp[:], AF.Copy, scale=ISC)
                if tq % 2 == 1:
                    i = tq // 2
                    nc.vector.tensor_tensor(
                        pmax[:, i * 512:(i + 1) * 512],
                        simcp[:, (tq - 1) * 512:tq * 512],
                        simcp[:, tq * 512:(tq + 1) * 512], ALU.max)
                if tq == 3 or tq == 7:
                    # dir1 half-merge + dir0 s-halving L1 (hide under MMs)
                    h = tq // 4
                    Th = work.tile([128, 512], BF16, tag=f"T{h}")
                    nc.vector.tensor_tensor(
                        Th[:], pmax[:, h * 1024:h * 1024 + 512],
                        pmax[:, h * 1024 + 512:h * 1024 + 1024], ALU.max)
                    if h == 0:
                        T1 = Th
                    else:
                        T2 = Th
                    sh = simcp[:, h * 2048:(h + 1) * 2048].rearrange(
                        "p (tk s) -> p tk s", s=T)
                    nc.vector.tensor_tensor(
                        Lh[:, h * 1024:(h + 1) * 1024].rearrange(
                            "p (tk s) -> p tk s", s=4),
                        sh[:, :, 0:4], sh[:, :, 4:8], ALU.max)
            Rm = work.tile([128, 512], BF16, tag="Rm")
            nc.vector.tensor_tensor(Rm[:], T1[:], T2[:], ALU.max)
            msum = work.tile([128, K], F32, tag="msum")
            nc.vector.tensor_reduce(msum[:],
                                    Rm[:].rearrange("p (k s) -> p k s", k=K),
                                    axis=AX.X, op=ALU.add)
            # dir0 L2/L3: (h,t,k,s4) -> (h,t,k)
            L2 = work.tile([128, 1024], BF16, tag="L2")
            lhv = Lh[:].rearrange("p (tk s) -> p tk s", s=4)
            nc.vector.tensor_tensor(
                L2[:].rearrange("p (tk s) -> p tk s", s=2),
                lhv[:, :, 0:2], lhv[:, :, 2:4], ALU.max)
            Am = work.tile([128, 512], BF16, tag="Am")
            l2v = L2[:].rearrange("p (tk s) -> p tk s", s=2)
            nc.vector.tensor_tensor(Am[:].rearrange("p (tk s) -> p tk s",
                                                    s=1),
                                    l2v[:, :, 0:1], l2v[:, :, 1:2], ALU.max)
            # asum tree over t: Am layout (h2, t4, k64)
            h1 = work.tile([128, 256], BF16, tag="h1")
            nc.vector.tensor_tensor(h1[:], Am[:, 0:256], Am[:, 256:512],
                                    ALU.add)
            h2 = work.tile([128, 128], BF16, tag="h2")
            nc.vector.tensor_tensor(h2[:], h1[:, 0:128], h1[:, 128:256],
                                    ALU.add)
            asum = work.tile([128, K], F32, tag="asum")
            nc.vector.tensor_tensor(asum[:], h2[:, 0:64], h2[:, 64:128],
                                    ALU.add)
            nc.vector.scalar_tensor_tensor(
                obuf_glo[:, g * K:(g + 1) * K], in0=asum[:], scalar=-16.0,
                in1=msum[:], op0=ALU.add, op1=ALU.add)
            nc.gpsimd.dma_start(oglo[:, g * K:(g + 1) * K],
                                obuf_glo[:, g * K:(g + 1) * K])

        def emit_seg(g):
            tsg_v = tsg_t[g][:].rearrange("p (j o w q) -> p j o w q", j=DJ,
                                          o=2, w=NW)
            segs = segpool.tile([64, NW * NW * 128], BF16, tag="segs")
            for v in range(NW):
                sp = psS.tile([64, NW * 128], F32, tag=f"sv{v}")
                for j in range(DJ):
                    nc.tensor.matmul(sp[:], sT_v[:, j, :, v, :],
                                     tsg_v[:, j], start=(j == 0),
                                     stop=(j == DJ - 1), perf_mode=DR)
                nc.scalar.activation(segs[:, v * 384:(v + 1) * 384], sp[:],
                                     AF.Copy, scale=ISC)
            # q2s = sum_w max_v  (contiguous 384-wide maxes over v-slices)
            m01 = work.tile([64, NW * 128], BF16, tag="m01")
            nc.vector.tensor_tensor(m01[:], segs[:, 0:384], segs[:, 384:768],
                                    ALU.max)
            m012 = work.tile([64, NW * 128], BF16, tag="m012")
            nc.vector.tensor_tensor(m012[:], m01[:], segs[:, 768:1152],
                                    ALU.max)
            ws = work.tile([64, 128], BF16, tag="ws")
            nc.vector.tensor_tensor(ws[:], m012[:, 0:128], m012[:, 128:256],
                                    ALU.add)
            nc.vector.scalar_tensor_tensor(
                obuf_seg[:, g * 256 + 128:g * 256 + 256], in0=ws[:],
                scalar=-3.0, in1=m012[:, 256:384], op0=ALU.add, op1=ALU.add)
            # s2q = sum_v max_w  (strided views over w, v in the free dim)
            vv = segs[:].rearrange("p (v w q) -> p v w q", v=NW, w=NW)
            W1 = work.tile([64, NW * 128], BF16, tag="W1")
            w1v = W1[:].rearrange("p (v q) -> p v q", v=NW)
            nc.vector.tensor_tensor(w1v, vv[:, :, 0, :], vv[:, :, 1, :],
                                    ALU.max)
            Wm = work.tile([64, NW * 128], BF16, tag="Wm")
            wmv = Wm[:].rearrange("p (v q) -> p v q", v=NW)
            nc.vector.tensor_tensor(wmv, w1v, vv[:, :, 2, :], ALU.max)
            vs = work.tile([64, 128], BF16, tag="vs")
            nc.vector.tensor_tensor(vs[:], Wm[:, 0:128], Wm[:, 128:256],
                                    ALU.add)
            nc.vector.scalar_tensor_tensor(
                obuf_seg[:, g * 256:g * 256 + 128], in0=vs[:], scalar=-3.0,
                in1=Wm[:, 256:384], op0=ALU.add, op1=ALU.add)
            nc.gpsimd.dma_start(oseg[:, g * 256:(g + 1) * 256],
                                obuf_seg[:, g * 256:(g + 1) * 256])

        # hybrid order: seg groups ride between mains so their trees hide
        # under main MMs; the tail is only seg3's short tree chain
        emit_main(0)
        emit_main(1)
        emit_seg(0)
        emit_main(2)
        emit_seg(1)
        emit_main(3)
        emit_seg(2)
        emit_seg(3)

    nc.compile()
    return nc


_NC_CACHE = None


def _get_nc():
    global _NC_CACHE
    if _NC_CACHE is None:
        _NC_CACHE = build_nc()
    return _NC_CACHE


# ------------------------------------------------------------------ host side
def _norm8(x, scale):
    n = np.sqrt((x * x).sum(-1, keepdims=True))
    n = np.maximum(n, 1e-12)
    return (scale * x / n).astype(NP_F8)


def build_in_maps(support_features, target_features, support_labels,
                  logit_scale, fusion_logits):
    support_features = np.asarray(support_features, dtype=np.float32)
    target_features = np.asarray(target_features, dtype=np.float32)
    support_labels = np.asarray(support_labels, dtype=np.int32)

    # ---- prototypes (exact f32 scatter-mean, normalized, x16, fp8)
    proto = np.zeros((K, T, D), np.float32)
    cnt = np.zeros((K,), np.float32)
    np.add.at(proto, support_labels % K, support_features)
    np.add.at(cnt, support_labels % K, 1.0)
    proto /= cnt[:, None, None]
    p8 = _norm8(proto, PSC)                                   # [K, T, D]
    segp = np.stack([proto[:, s:e].sum(1) for s, e in WINDOWS], 1)
    sp8 = _norm8(segp, PSC)                                   # [K, NW, D]

    # pT: [p][(j,o,k,s)]
    pT_h = np.ascontiguousarray(
        p8.reshape(K, T, DJ, 2, 128).transpose(4, 2, 3, 0, 1)
    ).reshape(128, DJ * 2 * K * T)
    # sT: [p][(j,o,v,k)]
    sT_h = np.ascontiguousarray(
        sp8.reshape(K, NW, DJ, 2, 128).transpose(4, 2, 3, 1, 0)
    ).reshape(128, DJ * 2 * NW * K)

    # ---- queries: normalized x64, fp8, d-major
    q8 = _norm8(target_features, QSC)                         # [Q, T, D]
    segq = np.stack([target_features[:, s:e].sum(1) for s, e in WINDOWS], 1)
    sq8 = _norm8(segq, QSC)                                   # [Q, NW, D]

    in_maps = []
    for c in range(NCORES):
        x8 = q8[c * QPC:(c + 1) * QPC]
        tf_h = np.ascontiguousarray(
            x8.reshape(G, 128, T, DJ, 2, 128).transpose(0, 5, 2, 3, 4, 1)
        ).reshape(G, 128, DJ * 2 * T * 128)
        s8 = sq8[c * QPC:(c + 1) * QPC]
        tsg_h = np.ascontiguousarray(
            s8.reshape(G, 128, NW, DJ, 2, 128).transpose(0, 5, 3, 4, 2, 1)
        ).reshape(G, 128, DJ * 2 * NW * 128)
        in_maps.append({"tf": tf_h, "tsg": tsg_h, "pT": pT_h, "sT": sT_h})
    return in_maps


def kernel(support_features, target_features, support_labels, logit_scale,
           fusion_logits):
    logit_scale = np.asarray(logit_scale, dtype=np.float32)
    fusion_logits = np.asarray(fusion_logits, dtype=np.float32)
    in_maps = build_in_maps(support_features, target_features, support_labels,
                            logit_scale, fusion_logits)
    nc = _get_nc()
    res = run_bass_kernel_spmd(nc, in_maps, core_ids=list(range(NCORES)))

    glo = np.empty((Q, K), np.float32)
    s2q = np.empty((Q, K), np.float32)
    q2s = np.empty((Q, K), np.float32)
    for c in range(NCORES):
        og = np.asarray(res.results[c]["oglo"]).reshape(128, G, K)
        glo[c * QPC:(c + 1) * QPC] = og.transpose(1, 0, 2).reshape(QPC, K)
        os_ = np.asarray(res.results[c]["oseg"]).reshape(64, G, 2, 128)
        # [k, g, which, q] -> [g, q, k]
        s2q[c * QPC:(c + 1) * QPC] = os_[:, :, 0].transpose(1, 2, 0).reshape(
            QPC, K)
        q2s[c * QPC:(c + 1) * QPC] = os_[:, :, 1].transpose(1, 2, 0).reshape(
            QPC, K)

    e = np.exp(fusion_logits - fusion_logits.max())
    fw = (e / e.sum()) * np.exp(logit_scale)
    fused = fw[0] * glo + fw[1] * s2q + fw[2] * q2s
    return (fused.astype(np.float32), glo, s2q, q2s)


if __name__ == "__main__":
    rng = np.random.default_rng(0)
    ins = {
        "support_features": rng.standard_normal((S, T, D), dtype=np.float32),
        "target_features": rng.standard_normal((Q, T, D), dtype=np.float32),
        "support_labels": (np.arange(S) % K).astype(np.int32),
        "logit_scale": np.float32(0.0),
        "fusion_logits": np.zeros(3, np.float32),
    }
    outs = kernel(**ins)
    for o in outs:
        print(o.shape, o.dtype, float(o.mean()))


# revision 20
# speedup vs baseline: 1.0371x; 1.0097x over previous
"""Trainium2 Bass kernel for few-shot video retrieval (bidirectional chamfer
distance to class prototypes, global frame-level + segment-level, fused).

Contract: kernel(**inputs) takes the FULL unsharded inputs (numpy) and returns
the full outputs (tuple of 4 [4096, 64] float32 arrays), matching reference().

Sharding: data-parallel over the query axis across 8 NeuronCores; prototypes
(computed on host, like the norm factors) replicated. Gather + fusion on host.

Device-side algorithm per core (512 queries = 4 slices of 128):
  - host pre-normalizes every query frame (x64) and every prototype frame
    (x16) in f32, then casts to fp8 e4m3 -> all PSUM results are 1024*sim
    with a single constant drain scale; no per-(q,t) norm factors on device
  - main sims GEMM: queries stationary (d-major), protoT moving, fp8
    DoubleRow (256-deep contraction), output [q, (k, ts)] with ts innermost
  - chamfer: dir0 (max over ts) = two grouped tensor_reduce ops (contiguous
    innermost axis, 2x bf16); dir1 (max over tq) = pairwise bf16 max TTs that
    pipeline with the PSUM drains; sums via strided reduces
  - segments: 3 separate GEMM groups (one per support window v), stationary
    = seg prototypes [d, k] so outputs land k-major on partitions 0-63;
    chamfer trees split between DVE and GpSimd
  - fusion softmax/exp + final gather/transpose on host
"""

import sys

sys.path.insert(0, "/opt/trn_rl_repo")

import numpy as np
import ml_dtypes
from contextlib import ExitStack

import concourse.bass as bass
import concourse.bacc as bacc
import concourse.tile as tile
from concourse import mybir
from concourse.bass_utils import run_bass_kernel_spmd

# ---------------------------------------------------------------- problem dims
S, Q, T, D = 256, 4096, 8, 1024
K = 64                      # classes
NCORES = 8
QPC = Q // NCORES           # 512 queries per core
G = QPC // 128              # 4 query-slices of 128 per core
DJ = 4                      # 4 DoubleRow chunks (256-deep)
NW = 3                      # segment windows
WINDOWS = ((0, 4), (2, 6), (4, 8))
QSC = 64.0                  # query fp8 scale (host-normalized frames)
PSC = 16.0                  # prototype fp8 scale
ISC = 1.0 / (QSC * PSC)     # drain scale: PSUM value = 1024 * sim

F32 = mybir.dt.float32
BF16 = mybir.dt.bfloat16
F8 = mybir.dt.float8e4
AF = mybir.ActivationFunctionType
ALU = mybir.AluOpType
AX = mybir.AxisListType
DR = mybir.MatmulPerfMode.DoubleRow

NP_F8 = ml_dtypes.float8_e4m3


# ---------------------------------------------------------------- bass kernel
def build_nc():
    nc = bacc.Bacc("TRN2", target_bir_lowering=False, debug=False,
                   num_devices=NCORES)

    # d-major normalized queries: [g][p(d%128)][(t, j, o, q)] (t-major so
    # each tq matmul group depends only on its own 1KB/partition slice)
    tf = nc.dram_tensor("tf", [G, 128, DJ * 2 * T * 128], F8,
                        kind="ExternalInput")
    # d-major normalized query segments: [g][p][(j, o, w, q)]
    tsg = nc.dram_tensor("tsg", [G, 128, DJ * 2 * NW * 128], F8,
                         kind="ExternalInput")
    # d-major normalized frame prototypes: [p][(j, o, k, s)]
    pT = nc.dram_tensor("pT", [128, DJ * 2 * K * T], F8, kind="ExternalInput")
    # d-major normalized segment prototypes: [p][(j, o, v, k)]
    sT = nc.dram_tensor("sT", [128, DJ * 2 * NW * K], F8,
                        kind="ExternalInput")

    # outputs: -global_dist q-major; -(s2q|q2s) k-major
    oglo = nc.dram_tensor("oglo", [128, G * K], F32, kind="ExternalOutput")
    oseg = nc.dram_tensor("oseg", [64, G * 2 * 128], F32,
                          kind="ExternalOutput")

    NWARM = 16

    with tile.TileContext(nc) as tc, ExitStack() as ctx:
        const = ctx.enter_context(tc.tile_pool(name="const", bufs=1))
        persist = ctx.enter_context(tc.tile_pool(name="persist", bufs=1))
        simpool = ctx.enter_context(tc.tile_pool(name="simpool", bufs=2))
        segpool = ctx.enter_context(tc.tile_pool(name="segpool", bufs=2))
        work = ctx.enter_context(tc.tile_pool(name="work", bufs=2))

        # ---------------- PE warmup burst (HAM ramp while inputs stream in)
        wz = const.tile([128, 256], F8)
        nc.gpsimd.memset(wz[:], 0)
        with tc.tile_pool(name="psW", bufs=1, space="PSUM") as psW:
            wps = psW.tile([128, 256], F32)
            for _ in range(NWARM):
                nc.tensor.matmul(wps[:], wz[:, 0:128], wz[:], start=True,
                                 stop=True)

        # ---------------- input DMAs (gpsimd queue wakes earliest; issue
        # order = priority order)
        pT_t = const.tile([128, DJ * 2 * K * T], F8)
        tf_t = []
        tsg_t = []
        for g in range(G):
            tf_t.append(const.tile([128, DJ * 2 * T * 128], F8,
                                   name=f"tf{g}"))
            tsg_t.append(const.tile([128, DJ * 2 * NW * 128], F8,
                                    name=f"tsg{g}"))
        # interleave pT j-chunks with tf0 t-chunks: the first matmul needs
        # only pT[j0] + tf0[t0-1]
        nc.gpsimd.dma_start(pT_t[:, 0:1024], pT[:, 0:1024])
        nc.gpsimd.dma_start(tf_t[0][:, 0:2048], tf[0][:, 0:2048])
        for h in range(1, 4):
            cols = slice(h * 1024, (h + 1) * 1024)
            nc.gpsimd.dma_start(pT_t[:, cols], pT[:, cols])
        for h in range(1, 4):
            cols = slice(h * 2048, (h + 1) * 2048)
            nc.gpsimd.dma_start(tf_t[0][:, cols], tf[0][:, cols])
        for g in range(1, G):
            for h in range(2):
                cols = slice(h * 4096, (h + 1) * 4096)
                nc.gpsimd.dma_start(tf_t[g][:, cols], tf[g][:, cols])
        sT_t = const.tile([128, DJ * 2 * NW * K], F8)
        nc.gpsimd.dma_start(sT_t[:], sT[:])
        for g in range(G):
            nc.gpsimd.dma_start(tsg_t[g][:], tsg[g])

        pT_v = pT_t[:].rearrange("p (j o k s) -> p j o k s", j=DJ, o=2, k=K)
        sT_v = sT_t[:].rearrange("p (j o v k) -> p j o v k", j=DJ, o=2, v=NW)

        obuf_glo = persist.tile([128, G * K], F32)
        obuf_seg = persist.tile([64, G * 2 * 128], F32)

        psM = ctx.enter_context(tc.tile_pool(name="psM", bufs=3,
                                             space="PSUM"))
        psS = ctx.enter_context(tc.tile_pool(name="psS", bufs=1,
                                             space="PSUM"))

        def emit_main(g):
            tf_v = tf_t[g][:].rearrange("p (t j o q) -> p t j o q", t=T,
                                        j=DJ, o=2)
            simcp = simpool.tile([128, T * K * T], BF16, tag="simcp")
            pmax = simpool.tile([128, 4 * K * T], BF16, tag="pmax")
            Lh = simpool.tile([128, 2 * 4 * K * 4], BF16, tag="Lh")
            for tq in range(T):
                mp = psM.tile([128, K * T], F32, tag="mp")
                for j in range(DJ):
                    nc.tensor.matmul(mp[:], tf_v[:, tq, j, :, :],
                                     pT_v[:, j], start=(j == 0),
                                     stop=(j == DJ - 1), perf_mode=DR)
                nc.scalar.activation(simcp[:, tq * 512:(tq + 1) * 512],
                                     mp[:], AF.Copy, scale=ISC)
                if tq % 2 == 1:
                    i = tq // 2
                    nc.vector.tensor_tensor(
                        pmax[:, i * 512:(i + 1) * 512],
                        simcp[:, (tq - 1) * 512:tq * 512],
                        simcp[:, tq * 512:(tq + 1) * 512], ALU.max)
                if tq == 3 or tq == 7:
                    # dir1 half-merge + dir0 s-halving L1 (hide under MMs)
                    h = tq // 4
                    Th = work.tile([128, 512], BF16, tag=f"T{h}")
                    nc.vector.tensor_tensor(
                        Th[:], pmax[:, h * 1024:h * 1024 + 512],
                        pmax[:, h * 1024 + 512:h * 1024 + 1024], ALU.max)
                    if h == 0:
                        T1 = Th
                    else:
                        T2 = Th
                    sh = simcp[:, h * 2048:(h + 1) * 2048].rearrange(
                        "p (tk s) -> p tk s", s=T)
                    nc.vector.tensor_tensor(
                        Lh[:, h * 1024:(h + 1) * 1024].rearrange(
                            "p (tk s) -> p tk s", s=4),
                        sh[:, :, 0:4], sh[:, :, 4:8], ALU.max)
            Rm = work.tile([128, 512], BF16, tag="Rm")
            nc.vector.tensor_tensor(Rm[:], T1[:], T2[:], ALU.max)
            msum = work.tile([128, K], F32, tag="msum")
            nc.vector.tensor_reduce(msum[:],
                                    Rm[:].rearrange("p (k s) -> p k s", k=K),
                                    axis=AX.X, op=ALU.add)
            # dir0 L2/L3: (h,t,k,s4) -> (h,t,k)
            L2 = work.tile([128, 1024], BF16, tag="L2")
            lhv = Lh[:].rearrange("p (tk s) -> p tk s", s=4)
            nc.vector.tensor_tensor(
                L2[:].rearrange("p (tk s) -> p tk s", s=2),
                lhv[:, :, 0:2], lhv[:, :, 2:4], ALU.max)
            Am = work.tile([128, 512], BF16, tag="Am")
            l2v = L2[:].rearrange("p (tk s) -> p tk s", s=2)
            nc.vector.tensor_tensor(Am[:].rearrange("p (tk s) -> p tk s",
                                                    s=1),
                                    l2v[:, :, 0:1], l2v[:, :, 1:2], ALU.max)
            # asum tree over t: Am layout (h2, t4, k64)
            h1 = work.tile([128, 256], BF16, tag="h1")
            nc.vector.tensor_tensor(h1[:], Am[:, 0:256], Am[:, 256:512],
                                    ALU.add)
            h2 = work.tile([128, 128], BF16, tag="h2")
            nc.vector.tensor_tensor(h2[:], h1[:, 0:128], h1[:, 128:256],
                                    ALU.add)
            asum = work.tile([128, K], F32, tag="asum")
            nc.vector.tensor_tensor(asum[:], h2[:, 0:64], h2[:, 64:128],
                                    ALU.add)
            nc.vector.scalar_tensor_tensor(
                obuf_glo[:, g * K:(g + 1) * K], in0=asum[:], scalar=-16.0,
                in1=msum[:], op0=ALU.add, op1=ALU.add)
            nc.gpsimd.dma_start(oglo[:, g * K:(g + 1) * K],
                                obuf_glo[:, g * K:(g + 1) * K])

        def emit_seg(g):
            tsg_v = tsg_t[g][:].rearrange("p (j o w q) -> p j o w q", j=DJ,
                                          o=2, w=NW)
            segs = segpool.tile([64, NW * NW * 128], BF16, tag="segs")
            for v in range(NW):
                sp = psS.tile([64, NW * 128], F32, tag=f"sv{v}")
                for j in range(DJ):
                    nc.tensor.matmul(sp[:], sT_v[:, j, :, v, :],
                                     tsg_v[:, j], start=(j == 0),
                                     stop=(j == DJ - 1), perf_mode=DR)
                nc.scalar.activation(segs[:, v * 384:(v + 1) * 384], sp[:],
                                     AF.Copy, scale=ISC)
            # q2s = sum_w max_v  (contiguous 384-wide maxes over v-slices)
            m01 = work.tile([64, NW * 128], BF16, tag="m01")
            nc.vector.tensor_tensor(m01[:], segs[:, 0:384], segs[:, 384:768],
                                    ALU.max)
            m012 = work.tile([64, NW * 128], BF16, tag="m012")
            nc.vector.tensor_tensor(m012[:], m01[:], segs[:, 768:1152],
                                    ALU.max)
            ws = work.tile([64, 128], BF16, tag="ws")
            nc.vector.tensor_tensor(ws[:], m012[:, 0:128], m012[:, 128:256],
                                    ALU.add)
            nc.vector.scalar_tensor_tensor(
                obuf_seg[:, g * 256 + 128:g * 256 + 256], in0=ws[:],
                scalar=-3.0, in1=m012[:, 256:384], op0=ALU.add, op1=ALU.add)
            # s2q = sum_v max_w  (strided views over w, v in the free dim)
            vv = segs[:].rearrange("p (v w q) -> p v w q", v=NW, w=NW)
            W1 = work.tile([64, NW * 128], BF16, tag="W1")
            w1v = W1[:].rearrange("p (v q) -> p v q", v=NW)
            nc.vector.tensor_tensor(w1v, vv[:, :, 0, :], vv[:, :, 1, :],
                                    ALU.max)
            Wm = work.tile([64, NW * 128], BF16, tag="Wm")
            wmv = Wm[:].rearrange("p (v q) -> p v q", v=NW)
            nc.vector.tensor_tensor(wmv, w1v, vv[:, :, 2, :], ALU.max)
            vs = work.tile([64, 128], BF16, tag="vs")
            nc.vector.tensor_tensor(vs[:], Wm[:, 0:128], Wm[:, 128:256],
                                    ALU.add)
            nc.vector.scalar_tensor_tensor(
                obuf_seg[:, g * 256:g * 256 + 128], in0=vs[:], scalar=-3.0,
                in1=Wm[:, 256:384], op0=ALU.add, op1=ALU.add)
            nc.gpsimd.dma_start(oseg[:, g * 256:(g + 1) * 256],
                                obuf_seg[:, g * 256:(g + 1) * 256])

        # hybrid order: seg groups ride between mains so their trees hide
        # under main MMs; the tail is only seg3's short tree chain
        emit_main(0)
        emit_main(1)
        emit_seg(0)
        emit_main(2)
        emit_seg(1)
        emit_seg(2)
        emit_main(3)
        emit_seg(3)

    nc.compile()
    return nc


_NC_CACHE = None


def _get_nc():
    global _NC_CACHE
    if _NC_CACHE is None:
        _NC_CACHE = build_nc()
    return _NC_CACHE


# ------------------------------------------------------------------ host side
def _norm8(x, scale):
    n = np.sqrt((x * x).sum(-1, keepdims=True))
    n = np.maximum(n, 1e-12)
    return (scale * x / n).astype(NP_F8)


def build_in_maps(support_features, target_features, support_labels,
                  logit_scale, fusion_logits):
    support_features = np.asarray(support_features, dtype=np.float32)
    target_features = np.asarray(target_features, dtype=np.float32)
    support_labels = np.asarray(support_labels, dtype=np.int32)

    # ---- prototypes (exact f32 scatter-mean, normalized, x16, fp8)
    proto = np.zeros((K, T, D), np.float32)
    cnt = np.zeros((K,), np.float32)
    np.add.at(proto, support_labels % K, support_features)
    np.add.at(cnt, support_labels % K, 1.0)
    proto /= cnt[:, None, None]
    p8 = _norm8(proto, PSC)                                   # [K, T, D]
    segp = np.stack([proto[:, s:e].sum(1) for s, e in WINDOWS], 1)
    sp8 = _norm8(segp, PSC)                                   # [K, NW, D]

    # pT: [p][(j,o,k,s)]
    pT_h = np.ascontiguousarray(
        p8.reshape(K, T, DJ, 2, 128).transpose(4, 2, 3, 0, 1)
    ).reshape(128, DJ * 2 * K * T)
    # sT: [p][(j,o,v,k)]
    sT_h = np.ascontiguousarray(
        sp8.reshape(K, NW, DJ, 2, 128).transpose(4, 2, 3, 1, 0)
    ).reshape(128, DJ * 2 * NW * K)

    # ---- queries: normalized x64, fp8, d-major
    q8 = _norm8(target_features, QSC)                         # [Q, T, D]
    segq = np.stack([target_features[:, s:e].sum(1) for s, e in WINDOWS], 1)
    sq8 = _norm8(segq, QSC)                                   # [Q, NW, D]

    in_maps = []
    for c in range(NCORES):
        x8 = q8[c * QPC:(c + 1) * QPC]
        tf_h = np.ascontiguousarray(
            x8.reshape(G, 128, T, DJ, 2, 128).transpose(0, 5, 2, 3, 4, 1)
        ).reshape(G, 128, DJ * 2 * T * 128)
        s8 = sq8[c * QPC:(c + 1) * QPC]
        tsg_h = np.ascontiguousarray(
            s8.reshape(G, 128, NW, DJ, 2, 128).transpose(0, 5, 3, 4, 2, 1)
        ).reshape(G, 128, DJ * 2 * NW * 128)
        in_maps.append({"tf": tf_h, "tsg": tsg_h, "pT": pT_h, "sT": sT_h})
    return in_maps


def kernel(support_features, target_features, support_labels, logit_scale,
           fusion_logits):
    logit_scale = np.asarray(logit_scale, dtype=np.float32)
    fusion_logits = np.asarray(fusion_logits, dtype=np.float32)
    in_maps = build_in_maps(support_features, target_features, support_labels,
                            logit_scale, fusion_logits)
    nc = _get_nc()
    res = run_bass_kernel_spmd(nc, in_maps, core_ids=list(range(NCORES)))

    glo = np.empty((Q, K), np.float32)
    s2q = np.empty((Q, K), np.float32)
    q2s = np.empty((Q, K), np.float32)
    for c in range(NCORES):
        og = np.asarray(res.results[c]["oglo"]).reshape(128, G, K)
        glo[c * QPC:(c + 1) * QPC] = og.transpose(1, 0, 2).reshape(QPC, K)
        os_ = np.asarray(res.results[c]["oseg"]).reshape(64, G, 2, 128)
        # [k, g, which, q] -> [g, q, k]
        s2q[c * QPC:(c + 1) * QPC] = os_[:, :, 0].transpose(1, 2, 0).reshape(
            QPC, K)
        q2s[c * QPC:(c + 1) * QPC] = os_[:, :, 1].transpose(1, 2, 0).reshape(
            QPC, K)

    e = np.exp(fusion_logits - fusion_logits.max())
    fw = (e / e.sum()) * np.exp(logit_scale)
    fused = fw[0] * glo + fw[1] * s2q + fw[2] * q2s
    return (fused.astype(np.float32), glo, s2q, q2s)


if __name__ == "__main__":
    rng = np.random.default_rng(0)
    ins = {
        "support_features": rng.standard_normal((S, T, D), dtype=np.float32),
        "target_features": rng.standard_normal((Q, T, D), dtype=np.float32),
        "support_labels": (np.arange(S) % K).astype(np.int32),
        "logit_scale": np.float32(0.0),
        "fusion_logits": np.zeros(3, np.float32),
    }
    outs = kernel(**ins)
    for o in outs:
        print(o.shape, o.dtype, float(o.mean()))
